# revision 2
# baseline (speedup 1.0000x reference)
"""GNN message passing + aggregation + linear projection on 8 TRN2 NeuronCores.

Reference computation:
    msgs = (features[pair_i] + features[pair_j]) * pair_w[:, None]   # [P, 128]
    agg  = segment_sum(msgs, node_idx, 50000)                        # [N, 128]
    out  = agg @ weight + bias                                       # [N, 128]

v3 strategy (streaming, no SWDGE gathers):
  - node space padded to 50176 = 8 cores x 49 windows x 128 nodes; node_idx
    sorted => each core owns a contiguous disjoint node range (no collective).
  - pairs are packed into 128-slot chunks per destination window (variable
    chunk count per window, ~1.5% padding).
  - HOST prepares two contiguous streams per core:
      G' [128, nch, 256] bf16: slot row = [f_i | f_j] (gathered on host)
      E' [128, nch, 128] bf16: E'[c, n] = w_slot * (rel_slot == n)
  - per chunk ONE matmul: psum[n, 0:256] += E'_chunk^T @ G'_chunk
    (E' stationary loads once; 256-wide moving covers both i and j halves).
  - window epilogue: DVE adds psum halves -> agg [n,d], PE transpose,
    ACT copy, PE GEMM with weight, DVE bias-add, ACT store.
  - engines: sync = G' loads, scalar(ACT) = E'/const loads + copies + stores,
    PE = matmuls, DVE = adds. GpSimd and SWDGE unused.
"""
import numpy as np
import ml_dtypes
import contextlib

import concourse.bass as bass
import concourse.mybir as mybir
from concourse import bacc
from concourse.bass_utils import run_bass_kernel_spmd

P = 128
N_NODES = 50000
NWIN = 392
NCORES = 8
WPC = NWIN // NCORES    # 49 windows per core
GB = 8                  # G' chunks per load tile
RG = 8                  # G' ring depth
EB = 8                  # E' chunks per load tile
RE = 8                  # E' ring depth


def _plan(node_idx, pair_w):
    """Chunk schedule shared by all cores (same compiled program).

    Windows are 128-node destination blocks. cw[w] = chunks for local window
    w = max over cores of ceil(count/128). Returns slot assignment arrays.
    """
    win = (node_idx // P).astype(np.int64)
    core = win // WPC
    wl = win - core * WPC
    key = core * WPC + wl
    counts = np.bincount(key, minlength=NCORES * WPC).reshape(NCORES, WPC)
    cw = np.ceil(counts.max(axis=0) / P).astype(np.int64)   # [49]
    nch = int(cw.sum())
    cbase = np.concatenate([[0], np.cumsum(cw)[:-1]])       # chunk base per w

    order = np.argsort(key, kind="stable")
    group_starts = np.searchsorted(key[order], np.arange(NCORES * WPC), side="left")
    rank = np.arange(len(order)) - group_starts[key[order]]
    core_s = core[order]
    wl_s = wl[order]
    slot = cbase[wl_s] * P + rank                            # slot within core

    tot = nch * P
    rel = np.zeros((NCORES, tot), dtype=np.int64)
    wv = np.zeros((NCORES, tot), dtype=np.float32)
    srt_i = np.zeros((NCORES, tot), dtype=np.int64)          # source rows
    srt_j = np.zeros((NCORES, tot), dtype=np.int64)
    filled = np.zeros((NCORES, tot), dtype=bool)

    lin = core_s * tot + slot
    rel.reshape(-1)[lin] = node_idx[order] - (core_s * WPC + wl_s) * P
    wv.reshape(-1)[lin] = pair_w[order]
    filled.reshape(-1)[lin] = True
    return dict(cw=cw, cbase=cbase, nch=nch), order, lin, rel, wv, srt_i, srt_j, filled


def _build(nch, cw):
    win_nchunks = [int(c) for c in cw]
    NGT = (nch + GB - 1) // GB
    NET = (nch + EB - 1) // EB

    nc = bacc.Bacc()
    dt = mybir.dt
    gp_d = nc.declare_dram_parameter("gp", [P, nch * 256], dt.bfloat16, isOutput=False)
    ep_d = nc.declare_dram_parameter("ep", [P, nch * P], dt.bfloat16, isOutput=False)
    ident_d = nc.declare_dram_parameter("ident", [P, P], dt.float32, isOutput=False)
    wt_d = nc.declare_dram_parameter("wt", [P, P], dt.float32, isOutput=False)
    bias_d = nc.declare_dram_parameter("biasb", [P, P], dt.float32, isOutput=False)
    out_d = nc.declare_dram_parameter("out", [WPC, P, P], dt.float32, isOutput=True)

    # chunk -> window end bookkeeping
    win_last_chunk = []
    g = 0
    last = -1
    for w in range(WPC):
        last = g + win_nchunks[w] - 1 if win_nchunks[w] > 0 else last
        g += win_nchunks[w]
        win_last_chunk.append(last)
    assert g == nch

    with (
        nc.Block() as block,
        contextlib.ExitStack() as st,
    ):
        sem = nc.semaphore
        gsems = [st.enter_context(sem(f"gsem{b}")) for b in range(RG)]
        esems = [st.enter_context(sem(f"esem{b}")) for b in range(RE)]
        csem = st.enter_context(sem("consts"))
        pe_c = st.enter_context(sem("pe_c"))    # chunks consumed
        act_a = st.enter_context(sem("act_a"))  # copy_a done
        dv_a = st.enter_context(sem("dv_a"))    # halves-add done
        pe_t = st.enter_context(sem("pe_t"))    # transpose done
        act_b = st.enter_context(sem("act_b"))  # copy_b done
        pe_g = st.enter_context(sem("pe_g"))    # gemm done
        dv_o = st.enter_context(sem("dv_o"))    # bias add done
        osem = st.enter_context(sem("osem"))    # store done

        sb = lambda name, shape, d_: st.enter_context(nc.sbuf_tensor(name, shape, d_))
        gp_t = [sb(f"gp{b}", [P, GB, 256], dt.bfloat16) for b in range(RG)]
        ep_t = [sb(f"ep{b}", [P, EB, P], dt.bfloat16) for b in range(RE)]
        ident_t = sb("ident_t", [P, P], dt.float32)
        wt_t = sb("wt_t", [P, P], dt.float32)
        bias_t = sb("bias_t", [P, P], dt.float32)
        agg_t = [sb(f"agg{b}", [P, P], dt.float32) for b in range(2)]
        agg0_t = [sb(f"agg0{b}", [P, P], dt.float32) for b in range(2)]
        aggT_t = [sb(f"aggT{b}", [P, P], dt.float32) for b in range(2)]
        out_t = [sb(f"out{b}", [P, P], dt.float32) for b in range(2)]

        ps = lambda name: st.enter_context(nc.psum_tensor(name, [P, 512], dt.float32))
        ps_agg = [ps(f"psagg{b}") for b in range(2)]
        ps_tr = [ps(f"pstr{b}") for b in range(2)]
        ps_gm = [ps(f"psgm{b}") for b in range(2)]

        @block.sync
        def _(eng: bass.BassEngine):
            # interleave G' and E' tile loads (same chunk cadence)
            assert GB == EB and NGT == NET
            for t in range(NGT):
                c0 = t * GB
                c1 = min(nch, c0 + GB)
                if t >= RG:
                    eng.wait_ge(pe_c, min(nch, (t - RG + 1) * GB))
                eng.dma_start(
                    gp_t[t % RG][:, : c1 - c0, :],
                    gp_d[:, c0 * 256 : c1 * 256],
                ).then_inc(gsems[t % RG], 16)
                if t >= RE:
                    eng.wait_ge(pe_c, min(nch, (t - RE + 1) * EB))
                eng.dma_start(
                    ep_t[t % RE][:, : c1 - c0, :],
                    ep_d[:, c0 * P : c1 * P],
                ).then_inc(esems[t % RE], 16)

        @block.scalar
        def _(ac: bass.BassScalarEngine):
            for ii, (dst_sb, src_d) in enumerate([
                (ident_t, ident_d), (wt_t, wt_d), (bias_t, bias_d),
            ]):
                ac.dma_start(dst_sb[:], src_d[:]).then_inc(csem, 16)

            def copy_a(w):
                # agg0_t[w%2] = ps_agg[w%2][:, 0:128] (first half to SBUF)
                if win_nchunks[w] > 0:
                    ac.wait_ge(pe_c, win_last_chunk[w] + 1)
                if w >= 2:
                    ac.wait_ge(dv_a, w - 1)  # agg0_t[w%2] consumed by halves_add(w-2)
                ac.activation(agg0_t[w % 2][:], ps_agg[w % 2][:, 0:P],
                              mybir.ActivationFunctionType.Copy).then_inc(act_a, 1)

            def copy_b(w):
                ac.wait_ge(pe_t, w + 1)
                if w >= 2:
                    ac.wait_ge(pe_g, w - 1)  # aggT_t[w%2] consumed by GEMM(w-2)
                ac.activation(aggT_t[w % 2][:], ps_tr[w % 2][:, 0:P],
                              mybir.ActivationFunctionType.Copy).then_inc(act_b, 1)

            def store(w):
                ac.wait_ge(dv_o, w + 1)
                if w >= 1:
                    ac.wait_ge(osem, 16 * w)
                ac.dma_start(out_d[w], out_t[w % 2][:]).then_inc(osem, 16)

            for w in range(WPC):
                copy_a(w)
                if w >= 1:
                    copy_b(w - 1)
                if w >= 2:
                    store(w - 2)
            copy_b(WPC - 1)
            store(WPC - 2)
            store(WPC - 1)
            ac.wait_ge(osem, 16 * WPC)

        @block.vector
        def _(dv: bass.BassVectorEngine):
            dv.wait_ge(csem, 48)

            def halves_add(w):
                dv.wait_ge(act_a, w + 1)  # agg0_t[w%2] written by copy_a(w)
                if w >= 2:
                    dv.wait_ge(pe_t, w - 1)  # agg_t[w%2] consumed by transpose(w-2)
                if win_nchunks[w] > 0:
                    dv.tensor_add(
                        out=agg_t[w % 2][:],
                        in0=ps_agg[w % 2][:, P : 2 * P],
                        in1=agg0_t[w % 2][:],
                    ).then_inc(dv_a, 1)
                else:
                    dv.tensor_scalar(
                        out=agg_t[w % 2][:],
                        in0=bias_t[:],
                        scalar1=0.0,
                        op0=mybir.AluOpType.mult,
                    ).then_inc(dv_a, 1)

            def bias_add(w):
                dv.wait_ge(pe_g, w + 1)
                if w >= 2:
                    dv.wait_ge(osem, 16 * (w - 1))
                dv.tensor_add(
                    out=out_t[w % 2][:],
                    in0=ps_gm[w % 2][:, 0:P],
                    in1=bias_t[:],
                ).then_inc(dv_o, 1)

            for w in range(WPC):
                halves_add(w)
                if w >= 2:
                    bias_add(w - 2)
            bias_add(WPC - 2)
            bias_add(WPC - 1)

        @block.tensor
        def _(pe: bass.BassTensorEngine):
            def epilogue_t(w):
                pe.wait_ge(dv_a, w + 1)
                if w >= 2:
                    pe.wait_ge(act_b, w - 1)  # ps_tr[w%2] drained by copy_b(w-2)
                pe.transpose(ps_tr[w % 2][:, 0:P], agg_t[w % 2][:],
                             ident_t[:]).then_inc(pe_t, 1)

            def epilogue_g(w):
                pe.wait_ge(act_b, w + 1)
                if w >= 2:
                    pe.wait_ge(dv_o, w - 1)  # ps_gm[w%2] consumed by bias-add(w-2)
                pe.matmul(
                    ps_gm[w % 2][:, 0:P],
                    lhsT=aggT_t[w % 2][:],
                    rhs=wt_t[:],
                    start=True,
                    stop=True,
                ).then_inc(pe_g, 1)

            g = 0
            for w in range(WPC):
                if w >= 2:
                    pe.wait_ge(dv_a, w - 1)  # ps_agg[w%2] drained by halves_add(w-2)
                nmm = win_nchunks[w]
                for k in range(nmm):
                    gt = g // GB
                    et = g // EB
                    pe.wait_ge(gsems[gt % RG], 16 * (gt // RG + 1))
                    pe.wait_ge(esems[et % RE], 16 * (et // RE + 1))
                    pe.matmul(
                        ps_agg[w % 2][:, 0:256],
                        lhsT=ep_t[et % RE][:, g - et * EB, :],
                        rhs=gp_t[gt % RG][:, g - gt * GB, :],
                        start=(k == 0),
                        stop=(k == nmm - 1),
                    ).then_inc(pe_c, 1)
                    g += 1
                if w >= 1:
                    epilogue_t(w - 1)
                if w >= 2:
                    epilogue_g(w - 2)
            epilogue_t(WPC - 1)
            epilogue_g(WPC - 2)
            epilogue_g(WPC - 1)

    nc.compile()
    return nc


def _host_arrays(plan, order, lin, rel, wv, filled, features, weight, bias,
                 pair_i, pair_j):
    nch = plan["nch"]
    tot = nch * P

    fbf = features.astype(ml_dtypes.bfloat16)
    ident = np.eye(P, dtype=np.float32)
    wt = weight.astype(np.float32)
    biasb = np.tile(bias.astype(np.float32), (P, 1))

    # scatter source rows per slot
    src_i = np.zeros(NCORES * tot, dtype=np.int64)
    src_j = np.zeros(NCORES * tot, dtype=np.int64)
    src_i[lin] = pair_i[order]
    src_j[lin] = pair_j[order]
    src_i = src_i.reshape(NCORES, tot)
    src_j = src_j.reshape(NCORES, tot)

    per_core = []
    for c in range(NCORES):
        gp = np.zeros((tot, 256), dtype=ml_dtypes.bfloat16)
        m = filled[c]
        gp[m, 0:128] = fbf[src_i[c][m]]
        gp[m, 128:256] = fbf[src_j[c][m]]
        # [slot, 256] -> [c, chunk, 256] -> [P, nch*256]
        gp = np.ascontiguousarray(
            gp.reshape(nch, P, 256).transpose(1, 0, 2)).reshape(P, nch * 256)

        ep = np.zeros(tot * P, dtype=np.float32)
        slots = np.nonzero(m)[0]
        # ep layout [P(slot%128), nch*128]: row c, col chunk*128 + rel
        cols = (slots // P) * P + rel[c][slots]
        rows = slots % P
        ep[rows * (nch * P) + cols] = wv[c][slots]
        ep = ep.reshape(P, nch * P).astype(ml_dtypes.bfloat16)

        per_core.append({
            "gp": gp, "ep": ep, "ident": ident, "wt": wt, "biasb": biasb,
        })
    return per_core


def _run(features, pair_w, weight, bias, pair_i, pair_j, node_idx, trace=False):
    features = np.asarray(features, dtype=np.float32)
    pair_w = np.asarray(pair_w, dtype=np.float32)
    weight = np.asarray(weight, dtype=np.float32)
    bias = np.asarray(bias, dtype=np.float32)
    pair_i = np.asarray(pair_i).astype(np.int64)
    pair_j = np.asarray(pair_j).astype(np.int64)
    node_idx_i = np.asarray(node_idx).astype(np.int64)

    plan, order, lin, rel, wv, _, _, filled = _plan(node_idx_i, pair_w)
    nc = _build(plan["nch"], plan["cw"])
    in_maps = _host_arrays(plan, order, lin, rel, wv, filled,
                           features, weight, bias, pair_i, pair_j)
    res = run_bass_kernel_spmd(nc, in_maps, list(range(NCORES)), trace=trace)
    outs = [np.asarray(res.results[c]["out"], dtype=np.float32).reshape(WPC * P, P)
            for c in range(NCORES)]
    full = np.concatenate(outs, axis=0)[:N_NODES]
    return full.astype(np.float32), res


def kernel(features, pair_w, weight, bias, pair_i, pair_j, node_idx):
    out, _ = _run(features, pair_w, weight, bias, pair_i, pair_j, node_idx)
    return out


def kernel_profiled(features, pair_w, weight, bias, pair_i, pair_j, node_idx):
    return _run(features, pair_w, weight, bias, pair_i, pair_j, node_idx,
                trace=True)


# revision 3
# speedup vs baseline: 1.1216x; 1.1216x over previous
"""GNN message passing + aggregation + linear projection on 8 TRN2 NeuronCores.

Reference computation:
    msgs = (features[pair_i] + features[pair_j]) * pair_w[:, None]   # [P, 128]
    agg  = segment_sum(msgs, node_idx, 50000)                        # [N, 128]
    out  = agg @ weight + bias                                       # [N, 128]

v3 strategy (streaming, no SWDGE gathers):
  - node space padded to 50176 = 8 cores x 49 windows x 128 nodes; node_idx
    sorted => each core owns a contiguous disjoint node range (no collective).
  - pairs are packed into 128-slot chunks per destination window (variable
    chunk count per window, ~1.5% padding).
  - HOST prepares two contiguous streams per core:
      G' [128, nch, 256] bf16: slot row = [f_i | f_j] (gathered on host)
      E' [128, nch, 128] bf16: E'[c, n] = w_slot * (rel_slot == n)
  - per chunk ONE matmul: psum[n, 0:256] += E'_chunk^T @ G'_chunk
    (E' stationary loads once; 256-wide moving covers both i and j halves).
  - window epilogue: DVE adds psum halves -> agg [n,d], PE transpose,
    ACT copy, PE GEMM with weight, DVE bias-add, ACT store.
  - engines: sync = G' loads, scalar(ACT) = E'/const loads + copies + stores,
    PE = matmuls, DVE = adds. GpSimd and SWDGE unused.
"""
import numpy as np
import ml_dtypes
import contextlib

import concourse.bass as bass
import concourse.mybir as mybir
from concourse import bacc
from concourse.bass_utils import run_bass_kernel_spmd

P = 128
N_NODES = 50000
NWIN = 392
NCORES = 8
WPC = NWIN // NCORES    # 49 windows per core
GB = 8                  # G' chunks per load tile
RG = 8                  # G' ring depth
EB = 8                  # E' chunks per load tile
RE = 8                  # E' ring depth


def _plan(node_idx, pair_w):
    """Chunk schedule shared by all cores (same compiled program).

    Windows are 128-node destination blocks. cw[w] = chunks for local window
    w = max over cores of ceil(count/128). Returns slot assignment arrays.
    """
    win = (node_idx // P).astype(np.int64)
    core = win // WPC
    wl = win - core * WPC
    key = core * WPC + wl
    counts = np.bincount(key, minlength=NCORES * WPC).reshape(NCORES, WPC)
    cw = np.ceil(counts.max(axis=0) / P).astype(np.int64)   # [49]
    nch = int(cw.sum())
    cbase = np.concatenate([[0], np.cumsum(cw)[:-1]])       # chunk base per w

    order = np.argsort(key, kind="stable")
    group_starts = np.searchsorted(key[order], np.arange(NCORES * WPC), side="left")
    rank = np.arange(len(order)) - group_starts[key[order]]
    core_s = core[order]
    wl_s = wl[order]
    slot = cbase[wl_s] * P + rank                            # slot within core

    tot = nch * P
    rel = np.zeros((NCORES, tot), dtype=np.int64)
    wv = np.zeros((NCORES, tot), dtype=np.float32)
    srt_i = np.zeros((NCORES, tot), dtype=np.int64)          # source rows
    srt_j = np.zeros((NCORES, tot), dtype=np.int64)
    filled = np.zeros((NCORES, tot), dtype=bool)

    lin = core_s * tot + slot
    rel.reshape(-1)[lin] = node_idx[order] - (core_s * WPC + wl_s) * P
    wv.reshape(-1)[lin] = pair_w[order]
    filled.reshape(-1)[lin] = True
    return dict(cw=cw, cbase=cbase, nch=nch), order, lin, rel, wv, srt_i, srt_j, filled


def _build(nch, cw, built):
    """built: bool array [nch]; True -> E' chunk built on DVE from rel/wv
    metadata, False -> streamed from DRAM."""
    win_nchunks = [int(c) for c in cw]
    NGT = (nch + GB - 1) // GB
    # streamed chunks only, in global order
    s_idx = [-1] * nch          # chunk -> index within streamed sequence
    b_idx = [-1] * nch          # chunk -> index within built sequence
    ns = nb = 0
    for g in range(nch):
        if built[g]:
            b_idx[g] = nb
            nb += 1
        else:
            s_idx[g] = ns
            ns += 1
    NET = (ns + EB - 1) // EB if ns else 0

    nc = bacc.Bacc()
    dt = mybir.dt
    gp_d = nc.declare_dram_parameter("gp", [P, nch * 256], dt.bfloat16, isOutput=False)
    ep_d = nc.declare_dram_parameter("ep", [P, max(1, ns * P)], dt.bfloat16, isOutput=False)
    relv_d = nc.declare_dram_parameter("relv", [P, max(1, nch)], dt.float32, isOutput=False)
    wvv_d = nc.declare_dram_parameter("wvv", [P, max(1, nch)], dt.float32, isOutput=False)
    iota_d = nc.declare_dram_parameter("iota", [P, P], dt.bfloat16, isOutput=False)
    ident_d = nc.declare_dram_parameter("ident", [P, P], dt.float32, isOutput=False)
    wt_d = nc.declare_dram_parameter("wt", [P, P], dt.float32, isOutput=False)
    bias_d = nc.declare_dram_parameter("biasb", [P, P], dt.float32, isOutput=False)
    out_d = nc.declare_dram_parameter("out", [WPC, P, P], dt.float32, isOutput=True)

    built_gid = [g for g in range(nch) if built[g]]
    # chunk -> window end bookkeeping
    win_last_chunk = []
    g = 0
    last = -1
    for w in range(WPC):
        last = g + win_nchunks[w] - 1 if win_nchunks[w] > 0 else last
        g += win_nchunks[w]
        win_last_chunk.append(last)
    assert g == nch

    with (
        nc.Block() as block,
        contextlib.ExitStack() as st,
    ):
        sem = nc.semaphore
        gsems = [st.enter_context(sem(f"gsem{b}")) for b in range(RG)]
        esems = [st.enter_context(sem(f"esem{b}")) for b in range(RE)]
        csem = st.enter_context(sem("consts"))
        pe_c = st.enter_context(sem("pe_c"))    # chunks consumed
        act_a = st.enter_context(sem("act_a"))  # copy_a done
        dv_a = st.enter_context(sem("dv_a"))    # halves-add done
        dvb = st.enter_context(sem("dvb"))      # built E' chunks done
        pe_t = st.enter_context(sem("pe_t"))    # transpose done
        act_b = st.enter_context(sem("act_b"))  # copy_b done
        pe_g = st.enter_context(sem("pe_g"))    # gemm done
        dv_o = st.enter_context(sem("dv_o"))    # bias add done
        osem = st.enter_context(sem("osem"))    # store done

        sb = lambda name, shape, d_: st.enter_context(nc.sbuf_tensor(name, shape, d_))
        gp_t = [sb(f"gp{b}", [P, GB, 256], dt.bfloat16) for b in range(RG)]
        ep_t = [sb(f"ep{b}", [P, EB, P], dt.bfloat16) for b in range(RE)]
        NBB = 12
        epb_t = [sb(f"epb{b}", [P, P], dt.bfloat16) for b in range(NBB)]
        relv_t = sb("relv_t", [P, max(1, nch)], dt.float32)
        wvv_t = sb("wvv_t", [P, max(1, nch)], dt.float32)
        iota_t = sb("iota_t", [P, P], dt.bfloat16)
        ident_t = sb("ident_t", [P, P], dt.float32)
        wt_t = sb("wt_t", [P, P], dt.float32)
        bias_t = sb("bias_t", [P, P], dt.float32)
        agg_t = [sb(f"agg{b}", [P, P], dt.float32) for b in range(2)]
        agg0_t = [sb(f"agg0{b}", [P, P], dt.float32) for b in range(2)]
        aggT_t = [sb(f"aggT{b}", [P, P], dt.float32) for b in range(2)]
        out_t = [sb(f"out{b}", [P, P], dt.float32) for b in range(2)]

        ps = lambda name: st.enter_context(nc.psum_tensor(name, [P, 512], dt.float32))
        ps_agg = [ps(f"psagg{b}") for b in range(2)]
        ps_tr = [ps(f"pstr{b}") for b in range(2)]
        ps_gm = [ps(f"psgm{b}") for b in range(2)]

        # for streamed-E' tile reuse: global chunk id of the LAST streamed
        # chunk in tile te; and the first global chunk id (issue ordering)
        s_last_g = [0] * NET
        s_first_g = [0] * NET
        for g in range(nch):
            si = s_idx[g]
            if si >= 0:
                te = si // EB
                s_last_g[te] = g
                if si % EB == 0:
                    s_first_g[te] = g

        # merged issue order: G' tile t first-chunk = t*GB; E' tile te
        # first-chunk = s_first_g[te]
        loads = [("g", t, t * GB) for t in range(NGT)] + \
                [("e", t, s_first_g[t]) for t in range(NET)]
        loads.sort(key=lambda x: (x[2], x[0]))

        @block.sync
        def _(eng: bass.BassEngine):
            eng.dma_start(relv_t[:, :], relv_d[:, :]).then_inc(csem, 16)
            eng.dma_start(wvv_t[:, :], wvv_d[:, :]).then_inc(csem, 16)
            eng.dma_start(iota_t[:], iota_d[:]).then_inc(csem, 16)
            for kind, t, _fg in loads:
                if kind == "g":
                    c0 = t * GB
                    c1 = min(nch, c0 + GB)
                    if t >= RG:
                        eng.wait_ge(pe_c, min(nch, (t - RG + 1) * GB))
                    eng.dma_start(
                        gp_t[t % RG][:, : c1 - c0, :],
                        gp_d[:, c0 * 256 : c1 * 256],
                    ).then_inc(gsems[t % RG], 16)
                else:
                    e0 = t * EB
                    e1 = min(ns, e0 + EB)
                    if t >= RE:
                        eng.wait_ge(pe_c, s_last_g[t - RE] + 1)
                    eng.dma_start(
                        ep_t[t % RE][:, : e1 - e0, :],
                        ep_d[:, e0 * P : e1 * P],
                    ).then_inc(esems[t % RE], 16)

        @block.scalar
        def _(ac: bass.BassScalarEngine):
            for ii, (dst_sb, src_d) in enumerate([
                (ident_t, ident_d), (wt_t, wt_d), (bias_t, bias_d),
            ]):
                ac.dma_start(dst_sb[:], src_d[:]).then_inc(csem, 16)

            def copy_a(w):
                # agg0_t[w%2] = ps_agg[w%2][:, 0:128] (first half to SBUF)
                if win_nchunks[w] > 0:
                    ac.wait_ge(pe_c, win_last_chunk[w] + 1)
                if w >= 2:
                    ac.wait_ge(dv_a, w - 1)  # agg0_t[w%2] consumed by halves_add(w-2)
                ac.activation(agg0_t[w % 2][:], ps_agg[w % 2][:, 0:P],
                              mybir.ActivationFunctionType.Copy).then_inc(act_a, 1)

            def copy_b(w):
                ac.wait_ge(pe_t, w + 1)
                if w >= 2:
                    ac.wait_ge(pe_g, w - 1)  # aggT_t[w%2] consumed by GEMM(w-2)
                ac.activation(aggT_t[w % 2][:], ps_tr[w % 2][:, 0:P],
                              mybir.ActivationFunctionType.Copy).then_inc(act_b, 1)

            def store(w):
                ac.wait_ge(dv_o, w + 1)
                if w >= 1:
                    ac.wait_ge(osem, 16 * w)
                ac.dma_start(out_d[w], out_t[w % 2][:]).then_inc(osem, 16)

            for w in range(WPC):
                copy_a(w)
                if w >= 1:
                    copy_b(w - 1)
                if w >= 2:
                    store(w - 2)
            copy_b(WPC - 1)
            store(WPC - 2)
            store(WPC - 1)
            ac.wait_ge(osem, 16 * WPC)

        # chunk ids per window (for DVE build scheduling)
        win_chunks = []
        g0 = 0
        for w in range(WPC):
            win_chunks.append(list(range(g0, g0 + win_nchunks[w])))
            g0 += win_nchunks[w]

        @block.vector
        def _(dv: bass.BassVectorEngine):
            dv.wait_ge(csem, 48 + 48)  # 3 consts on ACT + 3 metadata on sync

            def build(g):
                bi = b_idx[g]
                if bi >= NBB:
                    dv.wait_ge(pe_c, built_gid[bi - NBB] + 1)
                dv.tensor_scalar(
                    out=epb_t[bi % NBB][:],
                    in0=iota_t[:],
                    scalar1=relv_t[:, g : g + 1],
                    scalar2=wvv_t[:, g : g + 1],
                    op0=mybir.AluOpType.is_equal,
                    op1=mybir.AluOpType.mult,
                ).then_inc(dvb, 1)

            def halves_add(w):
                dv.wait_ge(act_a, w + 1)  # agg0_t[w%2] written by copy_a(w)
                if w >= 2:
                    dv.wait_ge(pe_t, w - 1)  # agg_t[w%2] consumed by transpose(w-2)
                if win_nchunks[w] > 0:
                    dv.tensor_add(
                        out=agg_t[w % 2][:],
                        in0=ps_agg[w % 2][:, P : 2 * P],
                        in1=agg0_t[w % 2][:],
                    ).then_inc(dv_a, 1)
                else:
                    dv.tensor_scalar(
                        out=agg_t[w % 2][:],
                        in0=bias_t[:],
                        scalar1=0.0,
                        op0=mybir.AluOpType.mult,
                    ).then_inc(dv_a, 1)

            def bias_add(w):
                dv.wait_ge(pe_g, w + 1)
                if w >= 2:
                    dv.wait_ge(osem, 16 * (w - 1))
                dv.tensor_add(
                    out=out_t[w % 2][:],
                    in0=ps_gm[w % 2][:, 0:P],
                    in1=bias_t[:],
                ).then_inc(dv_o, 1)

            for w in range(WPC):
                for g in win_chunks[w]:
                    if built[g]:
                        build(g)
                if w >= 1:
                    halves_add(w - 1)
                if w >= 3:
                    bias_add(w - 3)
            halves_add(WPC - 1)
            bias_add(WPC - 3)
            bias_add(WPC - 2)
            bias_add(WPC - 1)

        @block.tensor
        def _(pe: bass.BassTensorEngine):
            def epilogue_t(w):
                pe.wait_ge(dv_a, w + 1)
                if w >= 2:
                    pe.wait_ge(act_b, w - 1)  # ps_tr[w%2] drained by copy_b(w-2)
                pe.transpose(ps_tr[w % 2][:, 0:P], agg_t[w % 2][:],
                             ident_t[:]).then_inc(pe_t, 1)

            def epilogue_g(w):
                pe.wait_ge(act_b, w + 1)
                if w >= 2:
                    pe.wait_ge(dv_o, w - 1)  # ps_gm[w%2] consumed by bias-add(w-2)
                pe.matmul(
                    ps_gm[w % 2][:, 0:P],
                    lhsT=aggT_t[w % 2][:],
                    rhs=wt_t[:],
                    start=True,
                    stop=True,
                ).then_inc(pe_g, 1)

            g = 0
            for w in range(WPC):
                if w >= 2:
                    pe.wait_ge(dv_a, w - 1)  # ps_agg[w%2] drained by halves_add(w-2)
                nmm = win_nchunks[w]
                for k in range(nmm):
                    gt = g // GB
                    pe.wait_ge(gsems[gt % RG], 16 * (gt // RG + 1))
                    if built[g]:
                        bi = b_idx[g]
                        pe.wait_ge(dvb, bi + 1)
                        lhs = epb_t[bi % NBB][:]
                    else:
                        si = s_idx[g]
                        et = si // EB
                        pe.wait_ge(esems[et % RE], 16 * (et // RE + 1))
                        lhs = ep_t[et % RE][:, si - et * EB, :]
                    pe.matmul(
                        ps_agg[w % 2][:, 0:256],
                        lhsT=lhs,
                        rhs=gp_t[gt % RG][:, g - gt * GB, :],
                        start=(k == 0),
                        stop=(k == nmm - 1),
                    ).then_inc(pe_c, 1)
                    g += 1
                if w >= 1:
                    epilogue_t(w - 1)
                if w >= 2:
                    epilogue_g(w - 2)
            epilogue_t(WPC - 1)
            epilogue_g(WPC - 2)
            epilogue_g(WPC - 1)

    nc.compile()
    return nc


def _host_arrays(plan, order, lin, rel, wv, filled, features, weight, bias,
                 pair_i, pair_j, built):
    nch = plan["nch"]
    tot = nch * P

    fbf = features.astype(ml_dtypes.bfloat16)
    ident = np.eye(P, dtype=np.float32)
    wt = weight.astype(np.float32)
    biasb = np.tile(bias.astype(np.float32), (P, 1))
    iota = np.tile(np.arange(P, dtype=np.float32), (P, 1)).astype(ml_dtypes.bfloat16)
    streamed = ~built

    # scatter source rows per slot
    src_i = np.zeros(NCORES * tot, dtype=np.int64)
    src_j = np.zeros(NCORES * tot, dtype=np.int64)
    src_i[lin] = pair_i[order]
    src_j[lin] = pair_j[order]
    src_i = src_i.reshape(NCORES, tot)
    src_j = src_j.reshape(NCORES, tot)

    per_core = []
    for c in range(NCORES):
        gp = np.zeros((tot, 256), dtype=ml_dtypes.bfloat16)
        m = filled[c]
        gp[m, 0:128] = fbf[src_i[c][m]]
        gp[m, 128:256] = fbf[src_j[c][m]]
        # [slot, 256] -> [c, chunk, 256] -> [P, nch*256]
        gp = np.ascontiguousarray(
            gp.reshape(nch, P, 256).transpose(1, 0, 2)).reshape(P, nch * 256)

        ep = np.zeros(tot * P, dtype=np.float32)
        slots = np.nonzero(m)[0]
        # ep layout [P(slot%128), nch*128]: row c, col chunk*128 + rel
        cols = (slots // P) * P + rel[c][slots]
        rows = slots % P
        ep[rows * (nch * P) + cols] = wv[c][slots]
        ep = ep.reshape(P, nch, P)
        ep_s = np.ascontiguousarray(ep[:, streamed, :]).astype(ml_dtypes.bfloat16)
        ns = ep_s.shape[1]
        ep_s = ep_s.reshape(P, max(1, ns * P)) if ns else np.zeros((P, P), dtype=ml_dtypes.bfloat16)

        relv = np.ascontiguousarray(rel[c].reshape(nch, P).T).astype(np.float32)
        wvv = np.ascontiguousarray(wv[c].reshape(nch, P).T).astype(np.float32)

        per_core.append({
            "gp": gp, "ep": ep_s, "relv": relv, "wvv": wvv, "iota": iota,
            "ident": ident, "wt": wt, "biasb": biasb,
        })
    return per_core


def _run(features, pair_w, weight, bias, pair_i, pair_j, node_idx, trace=False):
    features = np.asarray(features, dtype=np.float32)
    pair_w = np.asarray(pair_w, dtype=np.float32)
    weight = np.asarray(weight, dtype=np.float32)
    bias = np.asarray(bias, dtype=np.float32)
    pair_i = np.asarray(pair_i).astype(np.int64)
    pair_j = np.asarray(pair_j).astype(np.int64)
    node_idx_i = np.asarray(node_idx).astype(np.int64)

    plan, order, lin, rel, wv, _, _, filled = _plan(node_idx_i, pair_w)
    nch = plan["nch"]
    built = (np.arange(nch) % 10) < 7        # 70% built on DVE, 30% streamed
    nc = _build(nch, plan["cw"], built)
    in_maps = _host_arrays(plan, order, lin, rel, wv, filled,
                           features, weight, bias, pair_i, pair_j, built)
    res = run_bass_kernel_spmd(nc, in_maps, list(range(NCORES)), trace=trace)
    outs = [np.asarray(res.results[c]["out"], dtype=np.float32).reshape(WPC * P, P)
            for c in range(NCORES)]
    full = np.concatenate(outs, axis=0)[:N_NODES]
    return full.astype(np.float32), res


def kernel(features, pair_w, weight, bias, pair_i, pair_j, node_idx):
    out, _ = _run(features, pair_w, weight, bias, pair_i, pair_j, node_idx)
    return out


def kernel_profiled(features, pair_w, weight, bias, pair_i, pair_j, node_idx):
    return _run(features, pair_w, weight, bias, pair_i, pair_j, node_idx,
                trace=True)
